# revision 1
# baseline (speedup 1.0000x reference)
"""Trainium2 Bass kernel for 2D block-local multi-head attention.

Problem (hardcoded): x [1,128,48,64] -> 3x3 conv projections to q/k/v
(d_model=32, 8 heads, d_head=4), t2t local_attention_2d with
query_shape=(128,24), memory_flange=(8,8), combine heads, 3x3 output conv.

Key structural facts exploited:
  * H=128, W=48, query blocks 128x24 -> exactly 2 blocks (nH=1, nW=2).
  * The memory flange (8 px each side) is entirely zero padding, which the
    reference masks with -1e9 (exp -> exactly 0 in fp32). So each block's
    effective key set is the static 128x32 strip of ORIGINAL pixels:
    block b queries = cols [24b, 24b+24), keys = cols [16b, 16b+32).
  * Softmax without max-subtraction is safe: logits are O(10), and bf16
    shares the fp32 exponent range, so exp cannot overflow.

Sharding: one head per NeuronCore (8 heads / 8 cores), zero cross-core
communication. Each core computes q/k/v for its head (full image), block-local
attention, and a partial output conv (contraction over its 4 head channels).
Host sums the 8 partial [64, 6144] results.

Conv trick: with channel-major tensors stored on the padded 130x50 grid, a
3x3 tap (dh, dw) is a pure flat-index shift of dh*50+dw, so the matmul RHS is
one contiguous run over padded output positions (matmul RHS must be 1-D);
the junk columns (c=48, 49) are dropped in the psum->SBUF copy. The output
conv additionally packs tap pairs (dh,0)+(dh,1) into one 8-partition
contraction using a copy of o^T pre-shifted by one column on partitions 4:8.

Attention layouts (channel-on-partition):
  logitsT psum [128 keys, G queries] = K_kt.T @ Q  (keys on partitions)
  exp tile (bf16) -> AV matmul:  av[8, q] += V'_kt.T @ exp_kt, where V'_kt
  [128 keys, 8] holds v in cols 0:4 and 1.0 in cols 4:8 (denominator rows).
Matmuls use float32r (full-rate fp32) for convs and bf16 for attention.
"""

import numpy as np

H, W, CIN, DM, NH, DH = 128, 48, 64, 32, 8, 4
HP, WP = H + 2, W + 2          # padded spatial dims for 3x3 SAME conv
PADN = HP * WP + 4             # padded flat buffer size (+4 tail overrun slack)
NPIX = H * W                   # 6144
QW, KW = 24, 32                # per-block query/key column widths
NQ = H * QW                    # 3072 queries per block
NK = H * KW                    # 4096 keys per block
NKT = 32                       # key tiles (128 keys each) per block
G = 1024                       # query granule (psum/ACT tile width)
NG = NQ // G                   # 3
CHUNK_ROWS = 8                 # conv output rows per matmul chunk
NCHUNK = H // CHUNK_ROWS       # 16
CN = CHUNK_ROWS * WP           # conv matmul free size (padded-width run), 400

_cached = {}


def _build_nc():
    import concourse.bacc as bacc
    import concourse.tile as tile
    import concourse.mybir as mybir

    f32 = mybir.dt.float32
    f32r = mybir.dt.float32r    # fp32 data, full-rate PE (reduced mul precision)
    bf16 = mybir.dt.bfloat16

    nc = bacc.Bacc("TRN2", target_bir_lowering=False)

    xx_d = nc.dram_tensor("xx", [128, PADN], bf16, kind="ExternalInput")
    wqkv_d = nc.dram_tensor("wqkv", [128, 6 * 12], bf16, kind="ExternalInput")
    bias_d = nc.dram_tensor("bias12", [12, 1], f32, kind="ExternalInput")
    wo2_d = nc.dram_tensor("wo2", [8, 3 * 64], f32r, kind="ExternalInput")
    wo1_d = nc.dram_tensor("wo1", [DH, 3 * 64], f32r, kind="ExternalInput")
    id4_d = nc.dram_tensor("id4", [DH, DH], bf16, kind="ExternalInput")
    zp_d = nc.dram_tensor("zp", [DH, PADN], f32r, kind="ExternalInput")
    outp_d = nc.dram_tensor("outp", [CIN, NPIX], f32, kind="ExternalOutput")

    with tile.TileContext(nc) as tc:
        with tc.tile_pool(name="main", bufs=1) as mp:
            xx = mp.tile([128, PADN], bf16)
            wqkv = mp.tile([128, 6 * 12], bf16)
            bias12 = mp.tile([12, 1], f32)
            wo2 = mp.tile([8, 3 * 64], f32r)
            wo1 = mp.tile([DH, 3 * 64], f32r)
            id4 = mp.tile([DH, DH], bf16)
            xx_ap = xx_d.ap()
            for q4 in range(4):
                s4 = (PADN // 4) * q4
                e4 = PADN if q4 == 3 else (PADN // 4) * (q4 + 1)
                nc.sync.dma_start(xx[:, s4:e4], xx_ap[:, s4:e4])
            nc.sync.dma_start(wqkv[:], wqkv_d.ap())
            nc.sync.dma_start(bias12[:], bias_d.ap())
            nc.sync.dma_start(wo2[:], wo2_d.ap())
            nc.sync.dma_start(wo1[:], wo1_d.ap())
            nc.sync.dma_start(id4[:], id4_d.ap())

            qkvT = mp.tile([12, NPIX], bf16)
            kTf = mp.tile([DH, NPIX], bf16)      # k^T spatial (DMA dest)
            vT = mp.tile([DH, NPIX], bf16)       # v^T spatial (DMA dest)
            kTb = mp.tile([DH, 2 * NK], bf16)    # block-contiguous key strips
            vTb = mp.tile([DH, 2 * NK], bf16)    # block-contiguous value strips
            qb = mp.tile([DH, 2 * NQ], bf16)     # block-contiguous queries
            vp = mp.tile([128, 2 * NKT * 8], bf16)  # V' tiles, ones in cols 4:8
            av_sb = mp.tile([8, 2 * NQ], f32)    # rows 0:4 unnorm o, 4:8 den
            ebias = mp.tile([128, 1], f32)       # exp input shift (overflow guard)
            actwarm = mp.tile([128, 1], f32)     # dummy exp target (table preload)
            pewarm = mp.tile([DH, 512], bf16)    # garbage src for PE HAM warmup
            den4 = mp.tile([DH, NQ], f32)        # per-block denominator staging
            oo = mp.tile([8, PADN], f32r)        # padded o^T; rows 4:8 = col+1

            # ---- q/k/v projections: 3x3 conv, tap pairs (dh,0)+(dh,1)
            # packed on 128 partitions (xx rows 64:128 are shifted by +1 col),
            # plus (dh,2) singles; bias added in the psum->SBUF copy.
            # Repacks/V' transposes are emitted per chunk, interleaved with the
            # conv, so the in-order DVE stream overlaps them with later chunks.
            nc.vector.memset(vp[:], 1.0)
            nc.vector.memset(ebias[:], -30.0)
            # dummy exp at t~0: pulls the ~2.7us ACT exp-table load off the
            # critical path (it would otherwise fire at the first real exp,
            # right when the attention pipeline starts)
            nc.scalar.activation(
                actwarm[:], ebias[:], mybir.ActivationFunctionType.Exp,
                bias=ebias[:],
            )
            # ~3.4us of dummy matmuls during the DMA-in window: drives the PE
            # HAM activity monitor to full clock (2.4GHz) before the conv, and
            # absorbs the cold-clock warmup in otherwise-idle PE time. Reads
            # uninitialized SBUF (never written - no deps), result unused.
            with tc.tile_pool(name="wps", bufs=1, space="PSUM") as wps:
                nc.vector.memset(pewarm[:], 1.0)
                wp = wps.tile([DH, 512], f32, tag="wp")
                for _ in range(6):
                    nc.tensor.matmul(wp[:], pewarm[:, 0:DH], pewarm[:],
                                     start=True, stop=True)
            qT_v = qkvT[0:4, :].rearrange("p (h w) -> p h w", w=W)
            qb_v = qb[:].rearrange("p (b h w) -> p b h w", b=2, w=QW)
            kT_v = kTf[:].rearrange("p (h w) -> p h w", w=W)
            kb_v = kTb[:].rearrange("p (b h w) -> p b h w", b=2, w=KW)
            vT_v = vT[:].rearrange("p (h w) -> p h w", w=W)
            vb_v = vTb[:].rearrange("p (b h w) -> p b h w", b=2, w=KW)
            with (
                tc.tile_pool(name="cps", bufs=2, space="PSUM") as cps,
                tc.tile_pool(name="tps", bufs=4, space="PSUM") as tps,
            ):
                for ci in range(NCHUNK):
                    ps = cps.tile([12, CN], f32, tag="cps")
                    f0 = ci * CHUNK_ROWS * WP
                    for dh in range(3):
                        s = f0 + dh * WP
                        nc.tensor.matmul(
                            ps[:], wqkv[:, 12 * dh:12 * (dh + 1)],
                            xx[:, s:s + CN],
                            start=(dh == 0), stop=False,
                        )
                        nc.tensor.matmul(
                            ps[:], wqkv[0:CIN, 36 + 12 * dh:36 + 12 * (dh + 1)],
                            xx[0:CIN, s + 2:s + 2 + CN],
                            start=False, stop=(dh == 2),
                        )
                    # bias add + drop the 2 junk columns (cast to bf16)
                    psv = ps[:].rearrange("p (r c) -> p r c", c=WP)
                    nc.vector.tensor_scalar_add(
                        qkvT[:, ci * CHUNK_ROWS * W:(ci + 1) * CHUNK_ROWS * W],
                        psv[:, :, 0:W], bias12[:])
                    r0 = ci * CHUNK_ROWS
                    rs = slice(r0 * W, (r0 + CHUNK_ROWS) * W)
                    nc.sync.dma_start(kTf[:, rs], qkvT[4:8, rs])
                    nc.sync.dma_start(vT[:, rs], qkvT[8:12, rs])
                    rr = slice(r0, r0 + CHUNK_ROWS)
                    for b in range(2):
                        nc.vector.tensor_copy(
                            qb_v[:, b, rr], qT_v[:, rr, QW * b:QW * b + QW])
                        nc.vector.tensor_copy(
                            kb_v[:, b, rr], kT_v[:, rr, 16 * b:16 * b + KW])
                        nc.vector.tensor_copy(
                            vb_v[:, b, rr], vT_v[:, rr, 16 * b:16 * b + KW])
                        for kt in (2 * ci, 2 * ci + 1):
                            ps2 = tps.tile([128, DH], bf16, tag="tps")
                            nc.tensor.transpose(
                                ps2[:],
                                vTb[:, b * NK + 128 * kt:b * NK + 128 * (kt + 1)],
                                id4[:],
                            )
                            base = (b * NKT + kt) * 8
                            nc.scalar.copy(vp[:, base:base + 4], ps2[:])

            # padded o^T borders zeroed while attention runs
            nc.sync.dma_start(oo[0:4, :], zp_d.ap())

            # ---- attention + per-block normalization ----
            oo_v = oo[0:DH, 0:HP * WP].rearrange("p (h w) -> p h w", w=WP)
            with (
                tc.tile_pool(name="lgp", bufs=3, space="PSUM") as lgp,
                tc.tile_pool(name="avp", bufs=1, space="PSUM") as avp,
                tc.tile_pool(name="exp", bufs=3) as exp_pool,
            ):
                def norm_half(b, hf):
                    # normalize rows [64*hf, 64*hf+64) of block b:
                    # o = unnorm / den, written into the padded o^T grid
                    HQ = NQ // 2
                    sl = slice(b * NQ + hf * HQ, b * NQ + (hf + 1) * HQ)
                    dn = den4[:, hf * HQ:(hf + 1) * HQ]
                    nc.sync.dma_start(dn, av_sb[4:8, sl])
                    nc.vector.reciprocal(dn, dn)
                    nc.vector.tensor_mul(av_sb[0:4, sl], av_sb[0:4, sl], dn)
                    on_v = av_sb[0:4, sl].rearrange("p (h w) -> p h w", w=QW)
                    r0 = hf * (H // 2)
                    nc.vector.tensor_copy(
                        oo_v[:, 1 + r0:1 + r0 + H // 2,
                             1 + QW * b:1 + QW * b + QW], on_v)

                for b in range(2):
                    for g in range(NG):
                        q0 = b * NQ + g * G
                        av = avp.tile([8, G], f32, tag="av")
                        for kt in range(NKT):
                            lg = lgp.tile([128, G], f32, tag="lg")
                            kap = kTb[:, b * NK + 128 * kt:b * NK + 128 * (kt + 1)]
                            for j in range(G // 512):
                                nc.tensor.matmul(
                                    lg[:, 512 * j:512 * (j + 1)],
                                    kap,
                                    qb[:, q0 + 512 * j:q0 + 512 * (j + 1)],
                                    start=True, stop=True,
                                )
                            ex = exp_pool.tile([128, G], bf16, tag="ex")
                            # bias shifts exp's overflow window to logits in
                            # (-57, +118) at zero cost (free affine stage);
                            # num/den scale identically so the result is exact
                            nc.scalar.activation(
                                ex[:], lg[:],
                                mybir.ActivationFunctionType.Exp,
                                bias=ebias[:],
                            )
                            vbase = (b * NKT + kt) * 8
                            for j in range(G // 512):
                                nc.tensor.matmul(
                                    av[:, 512 * j:512 * (j + 1)],
                                    vp[:, vbase:vbase + 8],
                                    ex[:, 512 * j:512 * (j + 1)],
                                    start=(kt == 0), stop=(kt == NKT - 1),
                                )
                        nc.vector.tensor_copy(av_sb[:, q0:q0 + G], av[:])
                        # rows [0,64) are covered by granules 0-1; rows
                        # [64,128) by granules 1-2 -> normalize early
                        if g == 1:
                            norm_half(b, 0)
                        elif g == 2:
                            norm_half(b, 1)

            # shifted copy for tap pairing: oo[4:8, c] = oo[0:4, c+1],
            # chunked by row-halves so the output conv can start early
            HF = (1 + H // 2) * WP
            nc.sync.dma_start(oo[4:8, 0:HF], oo[0:4, 1:HF + 1])
            nc.sync.dma_start(oo[4:8, HF:PADN - 1], oo[0:4, HF + 1:PADN])

            # ---- output conv (partial over this head's 4 channels) ----
            # tap pairs (dh,0)+(dh,1) via 8-partition contraction + (dh,2) singles
            outp_ap = outp_d.ap()
            with (
                tc.tile_pool(name="ops", bufs=2, space="PSUM") as ops,
                tc.tile_pool(name="ost", bufs=3) as ost,
            ):
                for ci in range(NCHUNK):
                    ps = ops.tile([CIN, CN], f32, tag="ops")
                    f0 = ci * CHUNK_ROWS * WP
                    for dh in range(3):
                        s = f0 + dh * WP
                        nc.tensor.matmul(
                            ps[:], wo2[:, 64 * dh:64 * (dh + 1)], oo[:, s:s + CN],
                            start=(dh == 0), stop=False,
                        )
                        nc.tensor.matmul(
                            ps[:], wo1[:, 64 * dh:64 * (dh + 1)],
                            oo[0:4, s + 2:s + 2 + CN],
                            start=False, stop=(dh == 2),
                        )
                    psv = ps[:].rearrange("p (r c) -> p r c", c=WP)
                    stage = ost.tile([CIN, CHUNK_ROWS * W], f32, tag="ost")
                    nc.vector.tensor_copy(stage[:], psv[:, :, 0:W])
                    nc.sync.dma_start(
                        outp_ap[:, ci * CHUNK_ROWS * W:(ci + 1) * CHUNK_ROWS * W],
                        stage[:])

    nc.compile()
    return nc


def _prep_inputs(x, wq, bq, wk, bk, wv, bv, wo):
    f32 = np.float32
    x = np.ascontiguousarray(np.asarray(x, f32))
    scale = f32(DH) ** -0.5

    bf = ml_bf16()
    xx = np.zeros((128, PADN), np.float32)
    xv = xx[:CIN, :HP * WP].reshape(CIN, HP, WP)
    xv[:, 1:1 + H, 1:1 + W] = x[0].transpose(2, 0, 1)
    xx[CIN:, :PADN - 1] = xx[:CIN, 1:]
    xx = xx.astype(bf)

    wq = np.asarray(wq, f32) * scale
    bq = np.asarray(bq, f32) * scale
    wk = np.asarray(wk, f32)
    bk = np.asarray(bk, f32)
    wv = np.asarray(wv, f32)
    bv = np.asarray(bv, f32)
    wo = np.asarray(wo, f32)

    id4 = np.eye(DH, dtype=ml_bf16())
    zp = np.zeros((DH, PADN), f32)
    in_maps = []
    for h in range(NH):
        sl = slice(4 * h, 4 * h + 4)
        wqkv = np.zeros((128, 6, 12), f32)
        for dh in range(3):
            for p, dw in ((0, 0), (1, 1)):   # pair slots on partition halves
                wqkv[64 * p:64 * p + CIN, dh, 0:4] = wq[dh, dw, :, sl]
                wqkv[64 * p:64 * p + CIN, dh, 4:8] = wk[dh, dw, :, sl]
                wqkv[64 * p:64 * p + CIN, dh, 8:12] = wv[dh, dw, :, sl]
            wqkv[:CIN, 3 + dh, 0:4] = wq[dh, 2, :, sl]
            wqkv[:CIN, 3 + dh, 4:8] = wk[dh, 2, :, sl]
            wqkv[:CIN, 3 + dh, 8:12] = wv[dh, 2, :, sl]
        bias12 = np.concatenate([bq[sl], bk[sl], bv[sl]]).reshape(12, 1)
        wo2 = np.zeros((8, 3, 64), f32)
        wo1 = np.zeros((DH, 3, 64), f32)
        for dh in range(3):
            wo2[0:4, dh] = wo[dh, 0, sl, :]
            wo2[4:8, dh] = wo[dh, 1, sl, :]
            wo1[:, dh] = wo[dh, 2, sl, :]
        in_maps.append({
            "xx": xx,
            "bias12": np.ascontiguousarray(bias12.astype(f32)),
            "wqkv": np.ascontiguousarray(wqkv.reshape(128, 6 * 12).astype(bf)),
            "wo2": np.ascontiguousarray(wo2.reshape(8, 3 * 64)),
            "wo1": np.ascontiguousarray(wo1.reshape(DH, 3 * 64)),
            "id4": id4,
            "zp": zp,
        })
    return in_maps


def ml_bf16():
    import ml_dtypes
    return ml_dtypes.bfloat16


def _run(in_maps, trace=False, trace_cores=None):
    from concourse.bass_utils import run_bass_kernel_spmd

    if "nc" not in _cached:
        _cached["nc"] = _build_nc()
    return run_bass_kernel_spmd(
        _cached["nc"], in_maps, core_ids=list(range(NH)),
        trace=trace, trace_cores=trace_cores,
    )


def kernel(x, wq, bq, wk, bk, wv, bv, wo):
    in_maps = _prep_inputs(x, wq, bq, wk, bk, wv, bv, wo)
    res = _run(in_maps)
    acc = np.zeros((CIN, NPIX), np.float64)
    for r in res.results:
        acc += r["outp"].astype(np.float64)
    out = acc.astype(np.float32).reshape(CIN, H, W).transpose(1, 2, 0)
    return out[None]



# revision 3
# speedup vs baseline: 1.2579x; 1.2579x over previous
"""Trainium2 Bass kernel for 2D block-local multi-head attention (v2).

Problem (hardcoded): x [1,128,48,64] -> 3x3 conv projections to q/k/v
(d_model=32, 8 heads, d_head=4), t2t local_attention_2d with
query_shape=(128,24), memory_flange=(8,8), combine heads, 3x3 output conv.

Structural facts (see reference): H=128, W=48 -> 2 query blocks (128x24);
the flange is all zero padding, so block b attends the static 128x32 strip
of real pixels: queries cols [24b,24b+24), keys cols [16b,16b+32).

Sharding: one head per NeuronCore (8 heads / 8 cores), no cross-core
communication. Each core computes its head's q/k/v conv, block-local
attention, and a partial output conv over its 4 channels; host sums the
8 partial [64, 6144] results.

v2 design - the baseline was ACT-bound (exp of 25.2M logits/core at
0.83ns/row ~= 200us). Key changes:
  * exp split across TWO engines: ACT computes exact Exp for ~17/32 key
    tiles per granule; DVE computes the rest with a one-instruction
    Schraudolph exp: int16 = round(logit * 128/ln2 + (127*128 + boff)),
    bitcast to bf16. The approximation's error is a smooth function of
    the logit, which softmax normalization largely cancels (measured
    end-to-end rel err 3.8e-3 vs 3.7e-3 for bf16).
  * AV uses exp-stationary matmuls: out[128q, 8] = ex_tile[128k,128q]^T
    @ V'_kt[128k, 8] accumulated over 32 key tiles into per-qtile psum
    accumulators sharing one psum bank (single start/stop group).
    V' holds v in cols 0:4 and 1.0 in cols 4:8 (softmax denominator).
  * qb/kb/vb block repacks done by strided sbuf->sbuf DMAs, not DVE.
  * output conv is a single 36-deep contraction (9 taps x 4 ch) per row
    chunk, reading a 36-partition oo buffer whose 9 row-blocks are
    tap-shifted copies of o^T scattered by DMA.
Engine budget: PE ~112us (logits 82 + convs 19 + AV 5 + misc), ACT/DVE
~110us each (exp + small staging), Pool: normalization muls + memsets.
"""

import numpy as np

H, W, CIN, DM, NH, DH = 128, 48, 64, 32, 8, 4
HP, WP = H + 2, W + 2          # padded spatial dims for 3x3 SAME conv
PADN = HP * WP + 4             # padded flat buffer size (+4 tail slack)
NPIX = H * W                   # 6144
QW, KW = 24, 32                # per-block query/key column widths
NQ = H * QW                    # 3072 queries per block
NK = H * KW                    # 4096 keys per block
NKT = 32                       # key tiles (128 keys each) per block
G = 1024                       # query granule (psum tile width)
NG = NQ // G                   # 3 granules per block
NQT = G // 128                 # 8 q-subtiles per granule
CHUNK_ROWS = 8                 # conv output rows per matmul chunk
NCHUNK = H // CHUNK_ROWS       # 16
CN = CHUNK_ROWS * WP           # conv matmul free size, 400
GUARD = 64                     # left guard in oo (negative tap shifts)
OO_N = GUARD + HP * WP + 8     # oo depth per partition (bf16 elems)
A_EXP = float((1 << 7) / np.log(2.0))    # 184.665 = 2^7 * log2(e)
B_EXP = float((127 << 7) - 5.5)          # exponent bias + mantissa tuning
# per-granule count of ACT-handled exp tiles (rest go to DVE Schraudolph)
ACT_TILES = 17

_cached = {}


def _act_set(nact):
    # interleave ACT/DVE assignments so both engines start immediately
    s = set(range(0, 2 * min(nact, 16), 2))
    extra = nact - len(s)
    odds = list(range(31, 0, -2))
    return s | set(odds[:extra])


def _build_nc():
    import concourse.bacc as bacc
    import concourse.tile as tile
    import concourse.mybir as mybir

    f32 = mybir.dt.float32
    bf16 = mybir.dt.bfloat16
    i16 = mybir.dt.int16

    nc = bacc.Bacc("TRN2", target_bir_lowering=False)

    xx_d = nc.dram_tensor("xx", [128, PADN], bf16, kind="ExternalInput")
    wqkv_d = nc.dram_tensor("wqkv", [128, 6 * 12], bf16, kind="ExternalInput")
    bias_d = nc.dram_tensor("bias12", [12, 1], f32, kind="ExternalInput")
    wo36_d = nc.dram_tensor("wo36", [36, 64], bf16, kind="ExternalInput")
    id4_d = nc.dram_tensor("id4", [DH, DH], bf16, kind="ExternalInput")
    id128_d = nc.dram_tensor("id128", [128, 128], bf16, kind="ExternalInput")
    outp_d = nc.dram_tensor("outp", [CIN, NPIX], f32, kind="ExternalOutput")

    ACT_SET = _act_set(ACT_TILES)

    with tile.TileContext(nc) as tc:
        with tc.tile_pool(name="main", bufs=1) as mp:
            xx = mp.tile([128, PADN], bf16)
            wqkv = mp.tile([128, 6 * 12], bf16)
            bias12 = mp.tile([12, 1], f32)
            wo36 = mp.tile([36, 64], bf16)
            id4 = mp.tile([DH, DH], bf16)
            id128 = mp.tile([128, 128], bf16)
            qkvT = mp.tile([12, NPIX], bf16)
            qb = mp.tile([DH, 2 * NQ], bf16)
            kb = mp.tile([DH, 2 * NK], bf16)
            vTb = mp.tile([DH, 2 * NK], bf16)
            vp = mp.tile([128, 2 * NKT * 8], bf16)   # V' tiles, ones in 4:8
            oT = mp.tile([DH, 2 * NQ], bf16)         # normalized o^T
            oo = mp.tile([36, OO_N], bf16)           # 9 tap-shifted o^T
            zbias = mp.tile([128, 1], f32)
            rec = mp.tile([128, NQT], f32)
            av_sb = mp.tile([128, NQT * 8], f32)
            o_sb = mp.tile([128, NQT * DH], bf16)
            actwarm = mp.tile([128, 1], f32)
            pewarm = mp.tile([DH, 512], bf16)

            xx_ap = xx_d.ap()
            for q4 in range(4):
                s4 = (PADN // 4) * q4
                e4 = PADN if q4 == 3 else (PADN // 4) * (q4 + 1)
                nc.sync.dma_start(xx[:, s4:e4], xx_ap[:, s4:e4])
            nc.sync.dma_start(wqkv[:], wqkv_d.ap())
            nc.sync.dma_start(bias12[:], bias_d.ap())
            nc.sync.dma_start(wo36[:], wo36_d.ap())
            nc.sync.dma_start(id4[:], id4_d.ap())
            nc.sync.dma_start(id128[:], id128_d.ap())

            # init memsets on otherwise-idle engines
            nc.gpsimd.memset(vp[:], 1.0)
            nc.gpsimd.memset(oo[:], 0.0)
            nc.vector.memset(zbias[:], 0.0)
            nc.vector.memset(pewarm[:], 1.0)
            # preload the ACT exp table off the critical path
            nc.scalar.activation(
                actwarm[:], zbias[:], mybir.ActivationFunctionType.Exp,
                bias=zbias[:],
            )
            # PE clock warmup: dummy matmuls on uninitialized SBUF
            with tc.tile_pool(name="wps", bufs=1, space="PSUM") as wps:
                wp = wps.tile([DH, 512], f32, tag="wp")
                for _ in range(6):
                    nc.tensor.matmul(wp[:], pewarm[:, 0:DH], pewarm[:],
                                     start=True, stop=True)

            # ---- q/k/v projections: 3x3 conv, tap pairs (dh,0)+(dh,1)
            # packed on 128 partitions (xx rows 64:128 are +1 col shifted),
            # plus (dh,2) singles; bias added in the DVE psum->SBUF copy.
            qkvT_v = qkvT[:].rearrange("p (h w) -> p h w", w=W)
            with tc.tile_pool(name="cps", bufs=2, space="PSUM") as cps:
                for ci in range(NCHUNK):
                    ps = cps.tile([12, CN], f32, tag="cps")
                    f0 = ci * CHUNK_ROWS * WP
                    for dh in range(3):
                        s = f0 + dh * WP
                        nc.tensor.matmul(
                            ps[:], wqkv[:, 12 * dh:12 * (dh + 1)],
                            xx[:, s:s + CN],
                            start=(dh == 0), stop=False,
                        )
                        nc.tensor.matmul(
                            ps[:], wqkv[0:CIN, 36 + 12 * dh:36 + 12 * (dh + 1)],
                            xx[0:CIN, s + 2:s + 2 + CN],
                            start=False, stop=(dh == 2),
                        )
                    psv = ps[:].rearrange("p (r c) -> p r c", c=WP)
                    nc.vector.tensor_scalar_add(
                        qkvT[:, ci * CHUNK_ROWS * W:(ci + 1) * CHUNK_ROWS * W],
                        psv[:, :, 0:W], bias12[:])
                    # block repacks via strided sbuf->sbuf DMA, per half-image
                    if ci in (NCHUNK // 2 - 1, NCHUNK - 1):
                        hf = 0 if ci == NCHUNK // 2 - 1 else 1
                        r0, r1 = hf * (H // 2), (hf + 1) * (H // 2)
                        HB = (H // 2)
                        for b in range(2):
                            nc.sync.dma_start(
                                qb[:, b * NQ + hf * HB * QW:
                                   b * NQ + (hf + 1) * HB * QW],
                                qkvT_v[0:4, r0:r1, QW * b:QW * b + QW])
                            nc.sync.dma_start(
                                kb[:, b * NK + hf * HB * KW:
                                   b * NK + (hf + 1) * HB * KW],
                                qkvT_v[4:8, r0:r1, 16 * b:16 * b + KW])
                            nc.sync.dma_start(
                                vTb[:, b * NK + hf * HB * KW:
                                    b * NK + (hf + 1) * HB * KW],
                                qkvT_v[8:12, r0:r1, 16 * b:16 * b + KW])

            # ---- V' build: per-kt transpose v^T [4,128] -> [128,4] on PE,
            # then one strided DVE copy per block into vp (ones preserved
            # in cols 4:8 from the memset).
            vp_v = vp[:].rearrange("p (t e) -> p t e", e=8)
            with tc.tile_pool(name="vps", bufs=1, space="PSUM") as vps:
                for b in range(2):
                    vpp = vps.tile([128, NKT * DH], bf16, tag="vpp")
                    for kt in range(NKT):
                        nc.tensor.transpose(
                            vpp[:, DH * kt:DH * (kt + 1)],
                            vTb[:, b * NK + 128 * kt:b * NK + 128 * (kt + 1)],
                            id4[:],
                        )
                    vpp_v = vpp[:].rearrange("p (t e) -> p t e", e=DH)
                    nc.vector.tensor_copy(
                        vp_v[:, b * NKT:(b + 1) * NKT, 0:DH], vpp_v[:])

            # ---- attention ----
            with (
                tc.tile_pool(name="lgp", bufs=3, space="PSUM") as lgp,
                tc.tile_pool(name="avs", bufs=1, space="PSUM") as avsp,
                tc.tile_pool(name="tps", bufs=1, space="PSUM") as tpsp,
                tc.tile_pool(name="exp", bufs=4) as exp_pool,
            ):
                for b in range(2):
                    for g in range(NG):
                        q0 = b * NQ + g * G
                        av = avsp.tile([128, NQT * 8], f32, tag="av")
                        exs = [None] * NKT

                        def emit_av(kt):
                            ex = exs[kt]
                            vbase = (b * NKT + kt) * 8
                            for qt in range(NQT):
                                nc.tensor.matmul(
                                    av[:, 8 * qt:8 * qt + 8],
                                    ex[:, 128 * qt:128 * (qt + 1)],
                                    vp[:, vbase:vbase + 8],
                                    start=(kt == 0 and qt == 0),
                                    stop=(kt == NKT - 1 and qt == NQT - 1),
                                    skip_group_check=True,
                                )

                        for kt in range(NKT):
                            lg = lgp.tile([128, G], f32, tag="lg")
                            kap = kb[:, b * NK + 128 * kt:b * NK + 128 * (kt + 1)]
                            for j in range(G // 512):
                                nc.tensor.matmul(
                                    lg[:, 512 * j:512 * (j + 1)],
                                    kap,
                                    qb[:, q0 + 512 * j:q0 + 512 * (j + 1)],
                                    start=True, stop=True,
                                )
                            ex = exp_pool.tile([128, G], bf16, tag="ex")
                            exs[kt] = ex
                            if kt in ACT_SET:
                                nc.scalar.activation(
                                    ex[:], lg[:],
                                    mybir.ActivationFunctionType.Exp,
                                    bias=zbias[:],
                                )
                            else:
                                nc.vector.tensor_scalar(
                                    ex[:].bitcast(i16), lg[:],
                                    A_EXP, B_EXP,
                                    mybir.AluOpType.mult, mybir.AluOpType.add)
                            # software-pipeline AV by 3 key tiles
                            if kt >= 3:
                                emit_av(kt - 3)
                        for kt in range(NKT - 3, NKT):
                            emit_av(kt)

                        # granule epilogue: normalize + transpose to o^T
                        nc.vector.tensor_copy(av_sb[:], av[:])
                        av_v = av_sb[:].rearrange("p (q e) -> p q e", e=8)
                        nc.vector.reciprocal(rec[:], av_v[:, :, 4])
                        tps = tpsp.tile([DH, G], bf16, tag="tps")
                        for qt in range(NQT):
                            nc.gpsimd.tensor_scalar(
                                o_sb[:, DH * qt:DH * (qt + 1)],
                                av_sb[:, 8 * qt:8 * qt + DH],
                                rec[:, qt:qt + 1], None,
                                mybir.AluOpType.mult)
                        for qt in range(NQT):
                            nc.tensor.transpose(
                                tps[:, 128 * qt:128 * (qt + 1)],
                                o_sb[:, DH * qt:DH * (qt + 1)],
                                id128[:],
                            )
                        nc.scalar.copy(oT[:, q0:q0 + G], tps[:])

                    # block epilogue: scatter o^T into the 9 tap-shifted
                    # row-blocks of oo (sbuf->sbuf DMAs, row-aligned)
                    oTb_v = oT[:, b * NQ:(b + 1) * NQ].rearrange(
                        "p (r c) -> p r c", c=QW)
                    for t in range(9):
                        dh, dw = t // 3, t % 3
                        off = GUARD + (1 - dh) * WP + (QW * b + 1 - dw)
                        dst = oo[4 * t:4 * t + 4, off:off + H * WP]
                        dst_v = dst.rearrange("p (r c) -> p r c", c=WP)
                        nc.sync.dma_start(dst_v[:, :, 0:QW], oTb_v[:])

            # ---- output conv: single 36-deep matmul per row chunk ----
            outp_ap = outp_d.ap()
            with (
                tc.tile_pool(name="ops", bufs=2, space="PSUM") as ops,
                tc.tile_pool(name="ost", bufs=2) as ost,
            ):
                for ci in range(NCHUNK):
                    ps = ops.tile([CIN, CN], f32, tag="ops")
                    nc.tensor.matmul(
                        ps[:], wo36[:],
                        oo[:, GUARD + ci * CHUNK_ROWS * WP:
                           GUARD + ci * CHUNK_ROWS * WP + CN],
                        start=True, stop=True,
                    )
                    psv = ps[:].rearrange("p (r c) -> p r c", c=WP)
                    stage = ost.tile([CIN, CHUNK_ROWS * W], f32, tag="ost")
                    if ci % 2 == 0:
                        nc.vector.tensor_copy(stage[:], psv[:, :, 0:W])
                    else:
                        nc.scalar.copy(stage[:], psv[:, :, 0:W])
                    nc.sync.dma_start(
                        outp_ap[:, ci * CHUNK_ROWS * W:(ci + 1) * CHUNK_ROWS * W],
                        stage[:])

    nc.compile()
    return nc


def ml_bf16():
    import ml_dtypes
    return ml_dtypes.bfloat16


def _prep_inputs(x, wq, bq, wk, bk, wv, bv, wo):
    f32 = np.float32
    x = np.ascontiguousarray(np.asarray(x, f32))
    scale = f32(DH) ** -0.5

    bf = ml_bf16()
    xx = np.zeros((128, PADN), np.float32)
    xv = xx[:CIN, :HP * WP].reshape(CIN, HP, WP)
    xv[:, 1:1 + H, 1:1 + W] = x[0].transpose(2, 0, 1)
    xx[CIN:, :PADN - 1] = xx[:CIN, 1:]
    xx = xx.astype(bf)

    wq = np.asarray(wq, f32) * scale
    bq = np.asarray(bq, f32) * scale
    wk = np.asarray(wk, f32)
    bk = np.asarray(bk, f32)
    wv = np.asarray(wv, f32)
    bv = np.asarray(bv, f32)
    wo = np.asarray(wo, f32)

    id4 = np.eye(DH, dtype=bf)
    id128 = np.eye(128, dtype=bf)
    in_maps = []
    for h in range(NH):
        sl = slice(4 * h, 4 * h + 4)
        wqkv = np.zeros((128, 6, 12), f32)
        for dh in range(3):
            for p, dw in ((0, 0), (1, 1)):   # pair slots on partition halves
                wqkv[64 * p:64 * p + CIN, dh, 0:4] = wq[dh, dw, :, sl]
                wqkv[64 * p:64 * p + CIN, dh, 4:8] = wk[dh, dw, :, sl]
                wqkv[64 * p:64 * p + CIN, dh, 8:12] = wv[dh, dw, :, sl]
            wqkv[:CIN, 3 + dh, 0:4] = wq[dh, 2, :, sl]
            wqkv[:CIN, 3 + dh, 4:8] = wk[dh, 2, :, sl]
            wqkv[:CIN, 3 + dh, 8:12] = wv[dh, 2, :, sl]
        bias12 = np.concatenate([bq[sl], bk[sl], bv[sl]]).reshape(12, 1)
        wo36 = np.zeros((36, 64), f32)
        for dh in range(3):
            for dw in range(3):
                wo36[(3 * dh + dw) * 4:(3 * dh + dw) * 4 + 4] = wo[dh, dw, sl, :]
        in_maps.append({
            "xx": xx,
            "bias12": np.ascontiguousarray(bias12.astype(f32)),
            "wqkv": np.ascontiguousarray(wqkv.reshape(128, 6 * 12).astype(bf)),
            "wo36": np.ascontiguousarray(wo36.astype(bf)),
            "id4": id4,
            "id128": id128,
        })
    return in_maps


def _run(in_maps, trace=False, trace_cores=None):
    from concourse.bass_utils import run_bass_kernel_spmd

    if "nc" not in _cached:
        _cached["nc"] = _build_nc()
    return run_bass_kernel_spmd(
        _cached["nc"], in_maps, core_ids=list(range(NH)),
        trace=trace, trace_cores=trace_cores,
    )


def kernel(x, wq, bq, wk, bk, wv, bv, wo):
    in_maps = _prep_inputs(x, wq, bq, wk, bk, wv, bv, wo)
    res = _run(in_maps)
    acc = np.zeros((CIN, NPIX), np.float64)
    for r in res.results:
        acc += r["outp"].astype(np.float64)
    out = acc.astype(np.float32).reshape(CIN, H, W).transpose(1, 2, 0)
    return out[None]


# revision 8
# speedup vs baseline: 1.3791x; 1.0963x over previous
"""Trainium2 Bass kernel for 2D block-local multi-head attention (v2.2).

Problem (hardcoded): x [1,128,48,64] -> 3x3 conv projections to q/k/v
(d_model=32, 8 heads, d_head=4), t2t local_attention_2d with
query_shape=(128,24), memory_flange=(8,8), combine heads, 3x3 output conv.

Structural facts (see reference): H=128, W=48 -> 2 query blocks (128x24);
the flange is all zero padding, so block b attends the static 128x32 strip
of real pixels: queries cols [24b,24b+24), keys cols [16b,16b+32).

Sharding: one head per NeuronCore (8 heads / 8 cores), no cross-core
communication. Each core computes its head's q/k/v conv, block-local
attention, and a partial output conv over its 4 channels; host sums the
8 partial [64, 6144] results.

Design notes - the v1 baseline was ACT-bound (exp of 25.2M logits/core at
0.83ns/row ~= 200us). Key structural changes:
  * exp split across TWO engines: ACT computes exact Exp for ~17/32 key
    tiles per granule; DVE computes the rest with a one-instruction
    Schraudolph exp: int16 = round(logit * 128/ln2 + (127*128 + boff)),
    bitcast to bf16. The approximation error is a smooth function of the
    logit, which softmax normalization largely cancels (measured
    end-to-end rel err 5.0e-3 vs 3.3e-3 for the bf16 baseline).
  * AV uses exp-stationary matmuls: out[128q, 8] = ex_tile[128k,128q]^T
    @ V'_kt[128k, 8] accumulated over 32 key tiles into per-qtile psum
    accumulators sharing one psum bank (single start/stop group).
    V' holds v in cols 0:4 and 1.0 in cols 4:8 (softmax denominator).
  * qb/kb/vb block repacks are strided sbuf->sbuf DMAs (not DVE), halved
    so attention on key tiles 0..15 starts while the conv's second half
    is still running (interleaved on PE; exp engines start ~12us in).
  * output conv is a single 36-deep contraction (9 taps x 4 ch) per row
    chunk, reading a 36-partition oo buffer whose row-blocks are
    tap-shifted copies of o^T scattered by DMA in two row-waves.
  * DMA issue is spread across the SP/ACT (HWDGE) and Pool (SWDGE)
    queues; HWDGE serializes ~0.65us per DMA so bursts matter.
"""

import numpy as np

H, W, CIN, DM, NH, DH = 128, 48, 64, 32, 8, 4
HP, WP = H + 2, W + 2          # padded spatial dims for 3x3 SAME conv
PADN = HP * WP + 4             # padded flat buffer size (+4 tail slack)
NPIX = H * W                   # 6144
QW, KW = 24, 32                # per-block query/key column widths
NQ = H * QW                    # 3072 queries per block
NK = H * KW                    # 4096 keys per block
NKT = 32                       # key tiles (128 keys each) per block
G = 1024                       # query granule (psum tile width)
NG = NQ // G                   # 3 granules per block
NQT = G // 128                 # 8 q-subtiles per granule
CHUNK_ROWS = 8                 # conv output rows per matmul chunk
NCHUNK = H // CHUNK_ROWS       # 16
CN = CHUNK_ROWS * WP           # conv matmul free size, 400
GUARD = 64                     # left guard in oo (negative tap shifts)
OO_N = GUARD + HP * WP + 8     # oo depth per partition (bf16 elems)
A_EXP = float((1 << 7) / np.log(2.0))    # 184.665 = 2^7 * log2(e)
B_EXP = float((127 << 7) - 5.5)          # exponent bias + mantissa tuning
ACT_TILES = 17                 # per-granule ACT-exp share (of 32)
AV_LAG = 3                     # AV trails logits by this many key tiles

_cached = {}


def _act_set(nact):
    # interleave ACT/DVE assignments so both engines start immediately
    s = set(range(0, 2 * min(nact, 16), 2))
    extra = nact - len(s)
    odds = list(range(31, 0, -2))
    return s | set(odds[:extra])


def _build_nc():
    import concourse.bacc as bacc
    import concourse.tile as tile
    import concourse.mybir as mybir

    f32 = mybir.dt.float32
    bf16 = mybir.dt.bfloat16
    i16 = mybir.dt.int16

    nc = bacc.Bacc("TRN2", target_bir_lowering=False)

    xx_d = nc.dram_tensor("xx", [128, PADN], bf16, kind="ExternalInput")
    wqkv_d = nc.dram_tensor("wqkv", [128, 6 * 12], bf16, kind="ExternalInput")
    bias_d = nc.dram_tensor("bias12", [12, 1], f32, kind="ExternalInput")
    wo36_d = nc.dram_tensor("wo36", [36, 64], bf16, kind="ExternalInput")
    id4_d = nc.dram_tensor("id4", [DH, DH], bf16, kind="ExternalInput")
    id128_d = nc.dram_tensor("id128", [128, 128], bf16, kind="ExternalInput")
    outp_d = nc.dram_tensor("outp", [CIN, NPIX], f32, kind="ExternalOutput")

    ACT_SET = _act_set(ACT_TILES)
    dma_rr = [0]

    with tile.TileContext(nc) as tc:
        def dma(dst, src):
            # spread DMA issue over the two HWDGE queues (SP, ACT)
            eng = (nc.sync, nc.scalar)[dma_rr[0] % 2]
            dma_rr[0] += 1
            return eng.dma_start(dst, src)

        with tc.tile_pool(name="main", bufs=1) as mp:
            xx = mp.tile([128, PADN], bf16)
            wqkv = mp.tile([128, 6 * 12], bf16)
            bias12 = mp.tile([12, 1], f32)
            wo36 = mp.tile([36, 64], bf16)
            id4 = mp.tile([DH, DH], bf16)
            id128 = mp.tile([128, 128], bf16)
            qkvT = mp.tile([12, NPIX], bf16)
            qb = mp.tile([DH, 2 * NQ], bf16)
            kb = mp.tile([DH, 2 * NK], bf16)
            vTb = mp.tile([DH, 2 * NK], bf16)
            vp = mp.tile([128, 2 * NKT * 8], bf16)   # V' tiles, ones in 4:8
            oT = mp.tile([DH, 2 * NQ], bf16)         # normalized o^T
            oo = mp.tile([36, OO_N], bf16)           # 9 tap-shifted o^T
            zbias = mp.tile([128, 1], f32)
            rec = mp.tile([128, NQT], f32)
            av_sb = mp.tile([128, NQT * 8], f32)
            o_sb = mp.tile([128, NQT * DH], bf16)
            actwarm = mp.tile([128, 1], f32)
            pewarm = mp.tile([DH, 512], bf16)

            nc.sync.dma_start(wqkv[:], wqkv_d.ap())
            nc.sync.dma_start(bias12[:], bias_d.ap())
            nc.scalar.dma_start(wo36[:], wo36_d.ap())
            nc.scalar.dma_start(id4[:], id4_d.ap())
            nc.scalar.dma_start(id128[:], id128_d.ap())
            xx_ap = xx_d.ap()
            for q4 in range(4):
                s4 = (PADN // 4) * q4
                e4 = PADN if q4 == 3 else (PADN // 4) * (q4 + 1)
                dma(xx[:, s4:e4], xx_ap[:, s4:e4])

            # init memsets on otherwise-idle engines
            nc.gpsimd.memset(vp[:], 1.0)
            nc.gpsimd.memset(oo[:], 0.0)
            nc.vector.memset(zbias[:], 0.0)
            nc.vector.memset(pewarm[:], 1.0)
            # preload the ACT exp table off the critical path
            nc.scalar.activation(
                actwarm[:], zbias[:], mybir.ActivationFunctionType.Exp,
                bias=zbias[:],
            )
            # PE clock warmup: dummy matmuls on uninitialized SBUF
            with tc.tile_pool(name="wps", bufs=1, space="PSUM") as wps:
                wp = wps.tile([DH, 512], f32, tag="wp")
                for _ in range(6):
                    nc.tensor.matmul(wp[:], pewarm[:, 0:DH], pewarm[:],
                                     start=True, stop=True)

            qkvT_v = qkvT[:].rearrange("p (h w) -> p h w", w=W)

            def repack(hf):
                # block repacks via strided sbuf->sbuf DMA, per half-image
                r0, r1 = hf * (H // 2), (hf + 1) * (H // 2)
                HB = H // 2
                for b in range(2):
                    dma(qb[:, b * NQ + hf * HB * QW:
                           b * NQ + (hf + 1) * HB * QW],
                        qkvT_v[0:4, r0:r1, QW * b:QW * b + QW])
                    dma(kb[:, b * NK + hf * HB * KW:
                           b * NK + (hf + 1) * HB * KW],
                        qkvT_v[4:8, r0:r1, 16 * b:16 * b + KW])
                    dma(vTb[:, b * NK + hf * HB * KW:
                            b * NK + (hf + 1) * HB * KW],
                        qkvT_v[8:12, r0:r1, 16 * b:16 * b + KW])

            vp_v = vp[:].rearrange("p (t e) -> p t e", e=8)

            def scatter(b, row0, row1):
                # write o^T rows [row0,row1) into the 9 tap-shifted
                # row-blocks of oo (sbuf->sbuf DMAs, row-aligned)
                oTb_v = oT[:, b * NQ:(b + 1) * NQ].rearrange(
                    "p (r c) -> p r c", c=QW)
                for t in range(9):
                    dh, dw = t // 3, t % 3
                    off = (GUARD + (1 - dh) * WP + (QW * b + 1 - dw)
                           + row0 * WP)
                    dst = oo[4 * t:4 * t + 4, off:off + (row1 - row0) * WP]
                    dst_v = dst.rearrange("p (r c) -> p r c", c=WP)
                    if t % 3 == 2:
                        nc.gpsimd.dma_start(
                            dst_v[:, :, 0:QW], oTb_v[:, row0:row1])
                    else:
                        dma(dst_v[:, :, 0:QW], oTb_v[:, row0:row1])

            # ---- q/k/v conv: 3x3, tap pairs (dh,0)+(dh,1) packed on 128
            # partitions (xx rows 64:128 are +1 col shifted) + (dh,2)
            # singles; bias added in the psum->SBUF staging copy
            # (alternating ACT/DVE so neither engine gates the cadence).
            with (
                tc.tile_pool(name="cps", bufs=3, space="PSUM") as cps,
                tc.tile_pool(name="vps", bufs=2, space="PSUM") as vps,
            ):
                for ci in range(NCHUNK):
                    ps = cps.tile([12, CN], f32, tag="cps")
                    f0 = ci * CHUNK_ROWS * WP
                    for dh in range(3):
                        s = f0 + dh * WP
                        nc.tensor.matmul(
                            ps[:], wqkv[:, 12 * dh:12 * (dh + 1)],
                            xx[:, s:s + CN],
                            start=(dh == 0), stop=False,
                        )
                        nc.tensor.matmul(
                            ps[:], wqkv[0:CIN, 36 + 12 * dh:36 + 12 * (dh + 1)],
                            xx[0:CIN, s + 2:s + 2 + CN],
                            start=False, stop=(dh == 2),
                        )
                    psv = ps[:].rearrange("p (r c) -> p r c", c=WP)
                    dst = qkvT[:, ci * CHUNK_ROWS * W:(ci + 1) * CHUNK_ROWS * W]
                    nc.vector.tensor_scalar_add(dst, psv[:, :, 0:W], bias12[:])
                    if ci in (NCHUNK // 2 - 1, NCHUNK - 1):
                        repack(1 if ci == NCHUNK - 1 else 0)
                        # V' tiles for this half's key tiles: PE transpose
                        # v^T [4,128] -> [128,4], strided DVE copy into vp
                        # (ones in cols 4:8 persist from the memset)
                        hf = 0 if ci == NCHUNK // 2 - 1 else 1
                        for b in range(2):
                            vpp = vps.tile([128, NKT * DH // 2], bf16,
                                           tag="vpp")
                            for i in range(16):
                                kt = 16 * hf + i
                                nc.tensor.transpose(
                                    vpp[:, DH * i:DH * (i + 1)],
                                    vTb[:, b * NK + 128 * kt:
                                        b * NK + 128 * (kt + 1)],
                                    id4[:],
                                )
                            vpp_v = vpp[:].rearrange("p (t e) -> p t e", e=DH)
                            nc.vector.tensor_copy(
                                vp_v[:, b * NKT + 16 * hf:
                                     b * NKT + 16 * hf + 16, 0:DH], vpp_v[:])

            # ---- attention main loop ----
            with (
                tc.tile_pool(name="lgp", bufs=3, space="PSUM") as lgp,
                tc.tile_pool(name="avs", bufs=1, space="PSUM") as avsp,
                tc.tile_pool(name="tps", bufs=1, space="PSUM") as tpsp,
                tc.tile_pool(name="exp", bufs=4) as exp_pool,
            ):
                for b in range(2):
                    for g in range(NG):
                        q0 = b * NQ + g * G
                        av = avsp.tile([128, NQT * 8], f32, tag="av")
                        exs = [None] * NKT

                        def emit_av(kt):
                            ex = exs[kt]
                            vbase = (b * NKT + kt) * 8
                            for qt in range(NQT):
                                nc.tensor.matmul(
                                    av[:, 8 * qt:8 * qt + 8],
                                    ex[:, 128 * qt:128 * (qt + 1)],
                                    vp[:, vbase:vbase + 8],
                                    start=(kt == 0 and qt == 0),
                                    stop=(kt == NKT - 1 and qt == NQT - 1),
                                    skip_group_check=True,
                                )

                        for kt in range(NKT):
                            lg = lgp.tile([128, G], f32, tag="lg")
                            kap = kb[:, b * NK + 128 * kt:
                                     b * NK + 128 * (kt + 1)]
                            for j in range(G // 512):
                                nc.tensor.matmul(
                                    lg[:, 512 * j:512 * (j + 1)],
                                    kap,
                                    qb[:, q0 + 512 * j:q0 + 512 * (j + 1)],
                                    start=True, stop=True,
                                )
                            ex = exp_pool.tile([128, G], bf16, tag="ex")
                            exs[kt] = ex
                            if kt in ACT_SET:
                                nc.scalar.activation(
                                    ex[:], lg[:],
                                    mybir.ActivationFunctionType.Exp,
                                    bias=zbias[:],
                                )
                            else:
                                nc.vector.tensor_scalar(
                                    ex[:].bitcast(i16), lg[:], A_EXP, B_EXP,
                                    mybir.AluOpType.mult, mybir.AluOpType.add)
                            if kt >= AV_LAG:
                                emit_av(kt - AV_LAG)
                        for kt in range(NKT - AV_LAG, NKT):
                            emit_av(kt)

                        # granule epilogue: normalize + transpose to o^T
                        nc.vector.tensor_copy(av_sb[:], av[:])
                        av_v = av_sb[:].rearrange("p (q e) -> p q e", e=8)
                        nc.vector.reciprocal(rec[:], av_v[:, :, 4])
                        tps = tpsp.tile([DH, G], bf16, tag="tps")
                        for qt in range(NQT):
                            nc.gpsimd.tensor_scalar(
                                o_sb[:, DH * qt:DH * (qt + 1)],
                                av_sb[:, 8 * qt:8 * qt + DH],
                                rec[:, qt:qt + 1], None,
                                mybir.AluOpType.mult)
                        for qt in range(NQT):
                            nc.tensor.transpose(
                                tps[:, 128 * qt:128 * (qt + 1)],
                                o_sb[:, DH * qt:DH * (qt + 1)],
                                id128[:],
                            )
                        nc.scalar.copy(oT[:, q0:q0 + G], tps[:])
                        if g == 1:
                            scatter(b, 0, 80)   # q rows 0:80 (1920 < 2048)
                        elif g == 2:
                            scatter(b, 80, H)

            # ---- output conv: single 36-deep matmul per row chunk;
            # chunks 0..8 only need oo rows < 80 (wave A of block 1).
            outp_ap = outp_d.ap()
            with (
                tc.tile_pool(name="ops", bufs=2, space="PSUM") as ops,
                tc.tile_pool(name="ost", bufs=2) as ost,
            ):
                BCH = 4   # chunks per staged output DMA
                for c0 in range(0, NCHUNK, BCH):
                    stage = ost.tile([CIN, BCH * CHUNK_ROWS * W], f32,
                                     tag="ost")
                    for ci in range(c0, c0 + BCH):
                        ps = ops.tile([CIN, CN], f32, tag="ops")
                        nc.tensor.matmul(
                            ps[:], wo36[:],
                            oo[:, GUARD + ci * CHUNK_ROWS * WP:
                               GUARD + ci * CHUNK_ROWS * WP + CN],
                            start=True, stop=True,
                        )
                        psv = ps[:].rearrange("p (r c) -> p r c", c=WP)
                        sl = slice((ci - c0) * CHUNK_ROWS * W,
                                   (ci - c0 + 1) * CHUNK_ROWS * W)
                        if ci % 2 == 0:
                            nc.vector.tensor_copy(stage[:, sl], psv[:, :, 0:W])
                        else:
                            nc.scalar.copy(stage[:, sl], psv[:, :, 0:W])
                    dma(outp_ap[:, c0 * CHUNK_ROWS * W:
                                (c0 + BCH) * CHUNK_ROWS * W], stage[:])

    nc.compile()
    return nc


def ml_bf16():
    import ml_dtypes
    return ml_dtypes.bfloat16


def _prep_inputs(x, wq, bq, wk, bk, wv, bv, wo):
    f32 = np.float32
    x = np.ascontiguousarray(np.asarray(x, f32))
    scale = f32(DH) ** -0.5

    bf = ml_bf16()
    xx = np.zeros((128, PADN), np.float32)
    xv = xx[:CIN, :HP * WP].reshape(CIN, HP, WP)
    xv[:, 1:1 + H, 1:1 + W] = x[0].transpose(2, 0, 1)
    xx[CIN:, :PADN - 1] = xx[:CIN, 1:]
    xx = xx.astype(bf)

    wq = np.asarray(wq, f32) * scale
    bq = np.asarray(bq, f32) * scale
    wk = np.asarray(wk, f32)
    bk = np.asarray(bk, f32)
    wv = np.asarray(wv, f32)
    bv = np.asarray(bv, f32)
    wo = np.asarray(wo, f32)

    id4 = np.eye(DH, dtype=bf)
    id128 = np.eye(128, dtype=bf)
    in_maps = []
    for h in range(NH):
        sl = slice(4 * h, 4 * h + 4)
        wqkv = np.zeros((128, 6, 12), f32)
        for dh in range(3):
            for p, dw in ((0, 0), (1, 1)):   # pair slots on partition halves
                wqkv[64 * p:64 * p + CIN, dh, 0:4] = wq[dh, dw, :, sl]
                wqkv[64 * p:64 * p + CIN, dh, 4:8] = wk[dh, dw, :, sl]
                wqkv[64 * p:64 * p + CIN, dh, 8:12] = wv[dh, dw, :, sl]
            wqkv[:CIN, 3 + dh, 0:4] = wq[dh, 2, :, sl]
            wqkv[:CIN, 3 + dh, 4:8] = wk[dh, 2, :, sl]
            wqkv[:CIN, 3 + dh, 8:12] = wv[dh, 2, :, sl]
        bias12 = np.concatenate([bq[sl], bk[sl], bv[sl]]).reshape(12, 1)
        wo36 = np.zeros((36, 64), f32)
        for dh in range(3):
            for dw in range(3):
                wo36[(3 * dh + dw) * 4:(3 * dh + dw) * 4 + 4] = wo[dh, dw, sl, :]
        in_maps.append({
            "xx": xx,
            "bias12": np.ascontiguousarray(bias12.astype(f32)),
            "wqkv": np.ascontiguousarray(wqkv.reshape(128, 6 * 12).astype(bf)),
            "wo36": np.ascontiguousarray(wo36.astype(bf)),
            "id4": id4,
            "id128": id128,
        })
    return in_maps


def _run(in_maps, trace=False, trace_cores=None):
    from concourse.bass_utils import run_bass_kernel_spmd

    if "nc" not in _cached:
        _cached["nc"] = _build_nc()
    return run_bass_kernel_spmd(
        _cached["nc"], in_maps, core_ids=list(range(NH)),
        trace=trace, trace_cores=trace_cores,
    )


def kernel(x, wq, bq, wk, bk, wv, bv, wo):
    in_maps = _prep_inputs(x, wq, bq, wk, bk, wv, bv, wo)
    res = _run(in_maps)
    acc = np.zeros((CIN, NPIX), np.float64)
    for r in res.results:
        acc += r["outp"].astype(np.float64)
    out = acc.astype(np.float32).reshape(CIN, H, W).transpose(1, 2, 0)
    return out[None]


# revision 15
# speedup vs baseline: 1.4710x; 1.0666x over previous
"""Trainium2 Bass kernel for 2D block-local multi-head attention (v3).

Problem (hardcoded): x [1,128,48,64] -> 3x3 conv projections to q/k/v
(d_model=32, 8 heads, d_head=4), t2t local_attention_2d with
query_shape=(128,24), memory_flange=(8,8), combine heads, 3x3 output conv.

Structural facts (see reference): H=128, W=48 -> 2 query blocks (128x24);
the flange is all zero padding, so block b attends the static 128x32 strip
of real pixels: queries cols [24b,24b+24), keys cols [16b,16b+32).

Sharding: one head per NeuronCore (8 heads / 8 cores), no cross-core
communication. Each core computes its head's q/k/v conv, block-local
attention, and a partial output conv over its 4 channels; host sums the
8 partial [64, 6144] results.

Design - the original baseline was ACT-bound (exp of 25.2M logits/core at
0.83 ns/row ~= 200us). Structural changes:
  * exp split across TWO engines: ACT computes exact Exp for ~17/32 key
    tiles per granule; DVE computes the rest with a one-instruction
    Schraudolph exp: int16 = round(logit * 128/ln2 + (127*128 + boff)),
    bitcast to bf16. The approximation error is a smooth function of the
    logit which softmax normalization largely cancels (measured
    end-to-end rel err 5.0e-3 vs 3.3e-3 for the bf16 baseline).
  * AV uses exp-stationary matmuls: out[128q, 8] = ex_tile[128k,128q]^T
    @ V'_kt[128k, 8] accumulated over 32 key tiles into per-qtile psum
    accumulators sharing one psum bank (single start/stop group).
    V' holds v in cols 0:4 and 1.0 in cols 4:8 (softmax denominator).
  * the attention is a single flattened stream of (block, granule,
    key-tile) units: logits(u_i) || exp(u_i) || AV(u_{i-4}), with the
    granule epilogue (normalize on DVE/Pool, PE transpose to o^T)
    emitted a few units late so no engine ever blocks on it.
  * granule (0,0)'s first 16 key tiles only need image rows 0:64 of
    k/q, so they interleave with conv chunks 8..15 - the exp engines
    start ~15us earlier (their AVs are flushed once V' is built).
  * qb/kb/vb block repacks are strided sbuf->sbuf DMAs, per half-image.
  * output conv is a single 36-deep contraction (9 taps x 4 ch) per row
    chunk reading a 36-partition oo buffer whose row-blocks are
    tap-shifted copies of o^T, DMA-scattered in two row-waves per block;
    chunks 0..7 run during the last granule, the rest at the end.
  * PSUM banks (8): lgp 3x2 + cps 2 (conv, freed) -> vps 1 (V' build,
    freed) -> ops 1 + avs 1 + ops2 1. DMA issue spread over SP/ACT
    HWDGE queues + Pool SWDGE (HWDGE serializes ~0.65us per DMA).
"""

import contextlib

import numpy as np

H, W, CIN, DM, NH, DH = 128, 48, 64, 32, 8, 4
HP, WP = H + 2, W + 2          # padded spatial dims for 3x3 SAME conv
PADN = HP * WP + 4             # padded flat buffer size (+4 tail slack)
NPIX = H * W                   # 6144
QW, KW = 24, 32                # per-block query/key column widths
NQ = H * QW                    # 3072 queries per block
NK = H * KW                    # 4096 keys per block
NKT = 32                       # key tiles (128 keys each) per block
G = 1024                       # query granule (psum tile width)
NG = NQ // G                   # 3 granules per block
NQT = G // 128                 # 8 q-subtiles per granule
CHUNK_ROWS = 8                 # conv output rows per matmul chunk
NCHUNK = H // CHUNK_ROWS       # 16
CN = CHUNK_ROWS * WP           # conv matmul free size, 400
GUARD = 64                     # left guard in oo (negative tap shifts)
OO_N = GUARD + HP * WP + 8     # oo depth per partition (bf16 elems)
A_EXP = float((1 << 7) / np.log(2.0))    # 184.665 = 2^7 * log2(e)
B_EXP = float((127 << 7) - 5.5)          # exponent bias + mantissa tuning
ACT_TILES = 17                 # per-granule ACT-exp share (of 32)
AV_LAG = 4                     # AV trails logits by this many stream units
# granule processing order: both blocks' granule 0 first (their first 16
# key tiles can interleave with the conv), then the rest; block scatter
# waves need g0+g1 (rows 0:80) resp. g2 (rows 80:128) of a block done.
SEQ = [(0, 0), (1, 0), (0, 1), (0, 2), (1, 1), (1, 2)]

_cached = {}


def _act_set(nact):
    # interleave ACT/DVE assignments so both engines start immediately
    s = set(range(0, 2 * min(nact, 16), 2))
    extra = nact - len(s)
    odds = list(range(31, 0, -2))
    return s | set(odds[:extra])


def _build_nc():
    import concourse.bacc as bacc
    import concourse.tile as tile
    import concourse.mybir as mybir

    f32 = mybir.dt.float32
    bf16 = mybir.dt.bfloat16
    i16 = mybir.dt.int16

    nc = bacc.Bacc("TRN2", target_bir_lowering=False)

    xx_d = nc.dram_tensor("xx", [128, PADN], bf16, kind="ExternalInput")
    wqkv_d = nc.dram_tensor("wqkv", [128, 6 * 12], bf16, kind="ExternalInput")
    bias_d = nc.dram_tensor("bias12", [12, 1], f32, kind="ExternalInput")
    wo36_d = nc.dram_tensor("wo36", [36, 64], bf16, kind="ExternalInput")
    id4_d = nc.dram_tensor("id4", [DH, DH], bf16, kind="ExternalInput")
    id128_d = nc.dram_tensor("id128", [128, 128], bf16, kind="ExternalInput")
    outp_d = nc.dram_tensor("outp", [CIN, NPIX], f32, kind="ExternalOutput")

    ACT_SET = _act_set(ACT_TILES)
    UNITS = [(b, g, kt) for (b, g) in SEQ for kt in range(NKT)]
    dma_rr = [0]

    with tile.TileContext(nc) as tc:
        stack = contextlib.ExitStack()

        def dma(dst, src, pool_ok=True):
            # spread DMA issue: SP and (startup only) ACT hwdge queues plus
            # the Pool swdge queue; ACT's queue is avoided mid-attention
            # since a DMA holds its SEQ ~1us, stalling the exp stream.
            engs = (nc.sync, nc.gpsimd) if pool_ok else (nc.sync, nc.scalar)
            eng = engs[dma_rr[0] % 2]
            dma_rr[0] += 1
            return eng.dma_start(dst, src)

        mp = stack.enter_context(tc.tile_pool(name="main", bufs=1))
        if True:
            xx = mp.tile([128, PADN], bf16)
            wqkv = mp.tile([128, 6 * 12], bf16)
            bias12 = mp.tile([12, 1], f32)
            wo36 = mp.tile([36, 64], bf16)
            id4 = mp.tile([DH, DH], bf16)
            id128 = mp.tile([128, 128], bf16)
            qkvT = mp.tile([12, NPIX], bf16)
            qb = mp.tile([DH, 2 * NQ], bf16)
            kb = mp.tile([DH, 2 * NK], bf16)
            vTb = mp.tile([DH, 2 * NK], bf16)
            vp = mp.tile([128, 2 * NKT * 8], bf16)   # V' tiles, ones in 4:8
            oT = mp.tile([DH, 2 * NQ], bf16)         # normalized o^T
            oo = mp.tile([36, OO_N], bf16)           # 9 tap-shifted o^T
            zbias = mp.tile([128, 1], f32)
            rec = mp.tile([128, NQT], f32)
            av_sb = mp.tile([128, NQT * 8], f32)
            o_sb = mp.tile([128, NQT * DH], bf16)
            actwarm = mp.tile([128, 1], f32)
            pewarm = mp.tile([DH, 512], bf16)

            nc.sync.dma_start(wqkv[:], wqkv_d.ap())
            nc.sync.dma_start(bias12[:], bias_d.ap())
            nc.scalar.dma_start(wo36[:], wo36_d.ap())
            nc.scalar.dma_start(id4[:], id4_d.ap())
            nc.scalar.dma_start(id128[:], id128_d.ap())
            xx_ap = xx_d.ap()
            for q4 in range(4):
                s4 = (PADN // 4) * q4
                e4 = PADN if q4 == 3 else (PADN // 4) * (q4 + 1)
                dma(xx[:, s4:e4], xx_ap[:, s4:e4], pool_ok=False)

            # init memsets on otherwise-idle engines
            nc.gpsimd.memset(vp[:], 1.0)
            nc.gpsimd.memset(oo[:], 0.0)
            nc.vector.memset(zbias[:], 0.0)
            nc.vector.memset(pewarm[:], 1.0)
            # preload the ACT exp table off the critical path
            nc.scalar.activation(
                actwarm[:], zbias[:], mybir.ActivationFunctionType.Exp,
                bias=zbias[:],
            )
            # PE clock warmup: dummy matmuls on uninitialized SBUF
            with tc.tile_pool(name="wps", bufs=1, space="PSUM") as wps:
                wp = wps.tile([DH, 512], f32, tag="wp")
                for _ in range(6):
                    nc.tensor.matmul(wp[:], pewarm[:, 0:DH], pewarm[:],
                                     start=True, stop=True)

            qkvT_v = qkvT[:].rearrange("p (h w) -> p h w", w=W)
            vp_v = vp[:].rearrange("p (t e) -> p t e", e=8)

            lgp = stack.enter_context(
                tc.tile_pool(name="lgp", bufs=3, space="PSUM"))
            exp_pool = stack.enter_context(tc.tile_pool(name="exp", bufs=20))
            ost = stack.enter_context(tc.tile_pool(name="ost", bufs=2))

            # ---- attention stream state ----
            st = {"avsp": None, "avt": {}, "exs": {}, "next_av": 0,
                  "done_g": []}

            def repack(hf):
                # block repacks via strided sbuf->sbuf DMA, per half-image
                r0, r1 = hf * (H // 2), (hf + 1) * (H // 2)
                HB = H // 2
                for b in range(2):
                    dma(qb[:, b * NQ + hf * HB * QW:
                           b * NQ + (hf + 1) * HB * QW],
                        qkvT_v[0:4, r0:r1, QW * b:QW * b + QW])
                    dma(kb[:, b * NK + hf * HB * KW:
                           b * NK + (hf + 1) * HB * KW],
                        qkvT_v[4:8, r0:r1, 16 * b:16 * b + KW])
                    dma(vTb[:, b * NK + hf * HB * KW:
                            b * NK + (hf + 1) * HB * KW],
                        qkvT_v[8:12, r0:r1, 16 * b:16 * b + KW])

            def emit_logits_exp(i):
                b, g, kt = UNITS[i]
                q0 = b * NQ + g * G
                lg = lgp.tile([128, G], f32, tag="lg")
                kap = kb[:, b * NK + 128 * kt:b * NK + 128 * (kt + 1)]
                for j in range(G // 512):
                    nc.tensor.matmul(
                        lg[:, 512 * j:512 * (j + 1)],
                        kap,
                        qb[:, q0 + 512 * j:q0 + 512 * (j + 1)],
                        start=True, stop=True,
                    )
                ex = exp_pool.tile([128, G], bf16, tag="ex")
                st["exs"][i] = ex
                if kt in ACT_SET:
                    nc.scalar.activation(
                        ex[:], lg[:], mybir.ActivationFunctionType.Exp,
                        bias=zbias[:],
                    )
                else:
                    nc.vector.tensor_scalar(
                        ex[:].bitcast(i16), lg[:], A_EXP, B_EXP,
                        mybir.AluOpType.mult, mybir.AluOpType.add)

            def emit_av(j):
                b, g, kt = UNITS[j]
                if (b, g) not in st["avt"]:
                    st["avt"][(b, g)] = st["avsp"].tile(
                        [128, NQT * 8], f32, tag="av", name=f"av_{b}_{g}")
                av = st["avt"][(b, g)]
                ex = st["exs"][j]
                vbase = (b * NKT + kt) * 8
                for qt in range(NQT):
                    nc.tensor.matmul(
                        av[:, 8 * qt:8 * qt + 8],
                        ex[:, 128 * qt:128 * (qt + 1)],
                        vp[:, vbase:vbase + 8],
                        start=(kt == 0 and qt == 0),
                        stop=(kt == NKT - 1 and qt == NQT - 1),
                        skip_group_check=True,
                    )
                if kt == NKT - 1:
                    epi1(b, g)
                st["exs"].pop(j)

            def epi1(b, g):
                # normalize o = num/den on DVE(recip)+Pool(muls)
                av = st["avt"].pop((b, g))
                nc.vector.tensor_copy(av_sb[:], av[:])
                av_v = av_sb[:].rearrange("p (q e) -> p q e", e=8)
                nc.vector.reciprocal(rec[:], av_v[:, :, 4])
                for qt in range(NQT):
                    nc.gpsimd.tensor_scalar(
                        o_sb[:, DH * qt:DH * (qt + 1)],
                        av_sb[:, 8 * qt:8 * qt + DH],
                        rec[:, qt:qt + 1], None,
                        mybir.AluOpType.mult)

            def epi2(b, g):
                # transpose [128q,4] -> [4,128q] into a bitcast view of a
                # spare lg psum slot, stage to o^T, fire scatter waves
                q0 = b * NQ + g * G
                tps_f32 = lgp.tile([128, G], f32, tag="lg")
                tps = tps_f32[0:DH, 0:G // 2].bitcast(bf16)
                for qt in range(NQT):
                    nc.tensor.transpose(
                        tps[:, 128 * qt:128 * (qt + 1)],
                        o_sb[:, DH * qt:DH * (qt + 1)],
                        id128[:],
                    )
                nc.scalar.copy(oT[:, q0:q0 + G], tps[:])
                st["done_g"].append((b, g))
                if (b, 0) in st["done_g"] and (b, 1) in st["done_g"] \
                        and g == 1:
                    scatter(b, 0, 80)
                elif g == 2:
                    scatter(b, 80, H)

            def scatter(b, row0, row1):
                # write o^T rows [row0,row1) into the 9 tap-shifted
                # row-blocks of oo (sbuf->sbuf DMAs, row-aligned)
                oTb_v = oT[:, b * NQ:(b + 1) * NQ].rearrange(
                    "p (r c) -> p r c", c=QW)
                for t in range(9):
                    dh, dw = t // 3, t % 3
                    off = (GUARD + (1 - dh) * WP + (QW * b + 1 - dw)
                           + row0 * WP)
                    dst = oo[4 * t:4 * t + 4, off:off + (row1 - row0) * WP]
                    dst_v = dst.rearrange("p (r c) -> p r c", c=WP)
                    dma(dst_v[:, :, 0:QW], oTb_v[:, row0:row1])

            def advance(i):
                # one stream step: logits+exp for unit i, AV for i-AV_LAG,
                # late epilogue-2 for a granule that ended at i-AV_LAG-2
                emit_logits_exp(i)
                while st["next_av"] <= i - AV_LAG:
                    emit_av(st["next_av"])
                    st["next_av"] += 1
                j2 = i - AV_LAG - 2
                if j2 >= 0 and UNITS[j2][2] == NKT - 1:
                    epi2(UNITS[j2][0], UNITS[j2][1])

            # ---- q/k/v conv: 3x3, tap pairs (dh,0)+(dh,1) packed on 128
            # partitions (xx rows 64:128 are +1 col shifted) + (dh,2)
            # singles; bias added in the DVE psum->SBUF staging copy.
            # Chunks 8..15 interleave with attention units 0..15 (granule
            # (0,0) key tiles 0..15 need only rows 0:64 of k/q).
            with tc.tile_pool(name="cps", bufs=2, space="PSUM") as cps:
                for ci in range(NCHUNK):
                    ps = cps.tile([12, CN], f32, tag="cps")
                    f0 = ci * CHUNK_ROWS * WP
                    for dh in range(3):
                        s = f0 + dh * WP
                        nc.tensor.matmul(
                            ps[:], wqkv[:, 12 * dh:12 * (dh + 1)],
                            xx[:, s:s + CN],
                            start=(dh == 0), stop=False,
                        )
                        nc.tensor.matmul(
                            ps[:], wqkv[0:CIN, 36 + 12 * dh:36 + 12 * (dh + 1)],
                            xx[0:CIN, s + 2:s + 2 + CN],
                            start=False, stop=(dh == 2),
                        )
                    psv = ps[:].rearrange("p (r c) -> p r c", c=WP)
                    nc.vector.tensor_scalar_add(
                        qkvT[:, ci * CHUNK_ROWS * W:(ci + 1) * CHUNK_ROWS * W],
                        psv[:, :, 0:W], bias12[:])
                    if ci == NCHUNK // 2 - 1:
                        repack(0)
                    elif ci >= NCHUNK // 2:
                        # 2 attention units between conv chunks (AV deferred
                        # via next_av until V' exists)
                        for u in (2 * (ci - NCHUNK // 2),
                                  2 * (ci - NCHUNK // 2) + 1):
                            emit_logits_exp(u)
                repack(1)

            # ---- V' build: per-kt PE transpose of v^T [4,128] -> [128,4],
            # one strided DVE copy per block into vp (ones in cols 4:8
            # persist from the memset)
            with tc.tile_pool(name="vps", bufs=2, space="PSUM") as vps:
                for b in range(2):
                    vpp = vps.tile([128, NKT * DH], bf16, tag="vpp")
                    for kt in range(NKT):
                        nc.tensor.transpose(
                            vpp[:, DH * kt:DH * (kt + 1)],
                            vTb[:, b * NK + 128 * kt:b * NK + 128 * (kt + 1)],
                            id4[:],
                        )
                    vpp_v = vpp[:].rearrange("p (t e) -> p t e", e=DH)
                    nc.vector.tensor_copy(
                        vp_v[:, b * NKT:(b + 1) * NKT, 0:DH], vpp_v[:])

            ops = stack.enter_context(
                tc.tile_pool(name="ops", bufs=1, space="PSUM"))
            outp_ap = outp_d.ap()
            oc_state = {"stage": None}

            def outconv_chunk(ci, pool, tag):
                ps = pool.tile([CIN, CN], f32, tag=tag)
                nc.tensor.matmul(
                    ps[:], wo36[:],
                    oo[:, GUARD + ci * CHUNK_ROWS * WP:
                       GUARD + ci * CHUNK_ROWS * WP + CN],
                    start=True, stop=True,
                )
                psv = ps[:].rearrange("p (r c) -> p r c", c=WP)
                if ci % 4 == 0:
                    oc_state["stage"] = ost.tile(
                        [CIN, 4 * CHUNK_ROWS * W], f32, tag="ost",
                        name=f"ostage_{ci}")
                stage = oc_state["stage"]
                sl = slice((ci % 4) * CHUNK_ROWS * W,
                           (ci % 4 + 1) * CHUNK_ROWS * W)
                if ci % 2 == 0:
                    nc.vector.tensor_copy(stage[:, sl], psv[:, :, 0:W])
                else:
                    nc.scalar.copy(stage[:, sl], psv[:, :, 0:W])
                if ci % 4 == 3:
                    dma(outp_ap[:, (ci - 3) * CHUNK_ROWS * W:
                                (ci + 1) * CHUNK_ROWS * W], stage[:])

            # ---- attention main stream; outconv chunks 0..7 interleave
            # with the last granule once both blocks' rows 0:80 are in oo.
            with tc.tile_pool(name="avs", bufs=1, space="PSUM") as avsp:
                st["avsp"] = avsp
                oc_early = iter(range(0, 8))
                for i in range(16, len(UNITS)):
                    advance(i)
                    if i >= 5 * NKT + 8 and i % 2 == 0:
                        ci = next(oc_early, None)
                        if ci is not None:
                            outconv_chunk(ci, ops, "opsA")
                # drain: AVs, last epilogue
                for j in range(st["next_av"], len(UNITS)):
                    emit_av(j)
                epi2(*SEQ[-1])

            # ---- output conv tail: remaining chunks, ping-pong between
            # the ops bank and the freed avs bank
            with tc.tile_pool(name="ops2", bufs=1, space="PSUM") as ops2:
                for ci in range(8, NCHUNK):
                    if ci % 2:
                        outconv_chunk(ci, ops2, "opsB")
                    else:
                        outconv_chunk(ci, ops, "opsA")

        stack.close()

    nc.compile()
    return nc


def ml_bf16():
    import ml_dtypes
    return ml_dtypes.bfloat16


def _prep_inputs(x, wq, bq, wk, bk, wv, bv, wo):
    f32 = np.float32
    x = np.ascontiguousarray(np.asarray(x, f32))
    scale = f32(DH) ** -0.5

    bf = ml_bf16()
    xx = np.zeros((128, PADN), np.float32)
    xv = xx[:CIN, :HP * WP].reshape(CIN, HP, WP)
    xv[:, 1:1 + H, 1:1 + W] = x[0].transpose(2, 0, 1)
    xx[CIN:, :PADN - 1] = xx[:CIN, 1:]
    xx = xx.astype(bf)

    wq = np.asarray(wq, f32) * scale
    bq = np.asarray(bq, f32) * scale
    wk = np.asarray(wk, f32)
    bk = np.asarray(bk, f32)
    wv = np.asarray(wv, f32)
    bv = np.asarray(bv, f32)
    wo = np.asarray(wo, f32)

    id4 = np.eye(DH, dtype=bf)
    id128 = np.eye(128, dtype=bf)
    in_maps = []
    for h in range(NH):
        sl = slice(4 * h, 4 * h + 4)
        wqkv = np.zeros((128, 6, 12), f32)
        for dh in range(3):
            for p, dw in ((0, 0), (1, 1)):   # pair slots on partition halves
                wqkv[64 * p:64 * p + CIN, dh, 0:4] = wq[dh, dw, :, sl]
                wqkv[64 * p:64 * p + CIN, dh, 4:8] = wk[dh, dw, :, sl]
                wqkv[64 * p:64 * p + CIN, dh, 8:12] = wv[dh, dw, :, sl]
            wqkv[:CIN, 3 + dh, 0:4] = wq[dh, 2, :, sl]
            wqkv[:CIN, 3 + dh, 4:8] = wk[dh, 2, :, sl]
            wqkv[:CIN, 3 + dh, 8:12] = wv[dh, 2, :, sl]
        bias12 = np.concatenate([bq[sl], bk[sl], bv[sl]]).reshape(12, 1)
        wo36 = np.zeros((36, 64), f32)
        for dh in range(3):
            for dw in range(3):
                wo36[(3 * dh + dw) * 4:(3 * dh + dw) * 4 + 4] = wo[dh, dw, sl, :]
        in_maps.append({
            "xx": xx,
            "bias12": np.ascontiguousarray(bias12.astype(f32)),
            "wqkv": np.ascontiguousarray(wqkv.reshape(128, 6 * 12).astype(bf)),
            "wo36": np.ascontiguousarray(wo36.astype(bf)),
            "id4": id4,
            "id128": id128,
        })
    return in_maps


def _run(in_maps, trace=False, trace_cores=None):
    from concourse.bass_utils import run_bass_kernel_spmd

    if "nc" not in _cached:
        _cached["nc"] = _build_nc()
    return run_bass_kernel_spmd(
        _cached["nc"], in_maps, core_ids=list(range(NH)),
        trace=trace, trace_cores=trace_cores,
    )


def kernel(x, wq, bq, wk, bk, wv, bv, wo):
    in_maps = _prep_inputs(x, wq, bq, wk, bk, wv, bv, wo)
    res = _run(in_maps)
    acc = np.zeros((CIN, NPIX), np.float64)
    for r in res.results:
        acc += r["outp"].astype(np.float64)
    out = acc.astype(np.float32).reshape(CIN, H, W).transpose(1, 2, 0)
    return out[None]
